# revision 11
# baseline (speedup 1.0000x reference)
"""Trainium2 kernel for nn_HandcraftedMultiplierV2.

Math notes (derived from the reference network's structure):
  - The attention stage collapses to a gather: the whole forward depends only
    on the 12 bits ids[b, 0:12].
  - For the actual parameter set the class total_int takes one of <=3
    consecutive values, reproduced exactly by an integer-weight linear
    threshold function of the bits (derived + verified over all 4096 patterns
    on the host at call time; integer arithmetic is exact in fp32 on device).
  - Output rows obey l0 = -l1 per position pair, and every output value
    ({0, +-0.5, +-9.5}) is exactly representable in bf16.

Device kernel (pure data parallel over 8 cores, t-last bf16 layout):
  score[b] = sum_i ids[b,i] * w_int[i]          (exact int32 dot, 12 cols)
  u1 = score >= T1, u2 = score >= T2            (bf16 0/1 masks, [128,TB])
  vc = b_tab*u1 + c_tab*u2                      (24-wide l1-value deltas)
  out[:, l, 1, t] = vc + a_tab                  (l1 values)
  out[:, l, 0, t] = na_tab - vc                 (l0 = -l1)
  All full-width ops are bf16 with packed innermost dims -> DVE 2x mode;
  output DMA is bf16 (half the bytes), host casts/transposes to f32 [B,L,2].
"""

import os
from contextlib import ExitStack

import numpy as np
import ml_dtypes

import concourse.bass as bass
import concourse.mybir as mybir
from concourse.bass_utils import run_bass_kernel_spmd

N_CORES = 8
B_FULL, L = 65536, 24
ROWS = B_FULL // N_CORES          # 8192 rows per core
TB = 32                           # batch rows per partition per block
TH = TB // 2                      # table replication shipped via DMA
NBLK = ROWS // (128 * TB)         # 2 blocks
NV = L                            # width of the l1-value (v) stage
NTAB = 4 * NV                     # a, b, c, na tables
F32 = mybir.dt.float32
BF16 = mybir.dt.bfloat16
I32 = mybir.dt.int32

_LAST = {}                        # exec_time_ns etc. for the test harness


# ----------------------------------------------------------------------------
# Host-side constant derivation (parameters only -- <10KB of data)
# ----------------------------------------------------------------------------

def _forward_totals(bits, emb, W_v, W_o, W1, b1, W2, b2):
    """fp32 `total` for each bit pattern, mirroring the reference arithmetic."""
    E = (emb.astype(np.float32) @ W_v.astype(np.float32).T)          # [2, 36]
    rep = np.repeat(np.arange(12), 3)                                # d -> head
    c = np.where(bits[:, rep] == 1, E[1][None, :], E[0][None, :]).astype(np.float32)
    attn = c @ W_o.astype(np.float32).T
    z = np.maximum(attn @ W1.astype(np.float32).T + b1.astype(np.float32), 0.0)
    mlp = z @ W2.astype(np.float32).T + b2.astype(np.float32)
    h2 = (attn + mlp).astype(np.float32)
    powers = np.exp2(np.arange(12)).astype(np.float32)
    return (h2[:, 12:24] * powers).sum(-1).astype(np.float32)


def _out_row(total_int):
    """The [L,2] output row for a given truncated total, flattened to [48]."""
    k = np.maximum(np.arange(L), 11) - 11
    ki = np.minimum(k, 11)
    m = k < 12
    bit = ((int(total_int) >> ki) & 1).astype(np.float32)
    l1 = np.where(m, bit * 10.0 - 0.5, 0.0)
    l0 = np.where(m, -bit * 10.0 + 0.5, 0.0)
    return np.stack([l0, l1], -1).reshape(2 * L).astype(np.float32)


def _derive_constants(emb, W_v, W_o, W1, b1, W2, b2):
    pat = np.arange(4096)
    bits = ((pat[:, None] >> np.arange(12)) & 1).astype(np.int64)    # [4096, 12]
    total = _forward_totals(bits, emb, W_v, W_o, W1, b1, W2, b2)
    lab = total.astype(np.int32)                                     # class per pattern
    classes = np.unique(lab)
    if len(classes) > 3:
        raise RuntimeError(f"expected <=3 classes, got {classes}")

    # Integer linear threshold reproducing `lab` exactly over all 4096 patterns.
    A = np.hstack([bits.astype(np.float64), np.ones((4096, 1))])
    coef, *_ = np.linalg.lstsq(A, total.astype(np.float64), rcond=None)
    w_real = coef[:12]

    def try_weights(w_int):
        s = bits @ w_int                                             # exact ints
        thr = []
        for lo_c, hi_c in zip(classes[:-1], classes[1:]):
            lo = s[lab == lo_c].max()
            hi = s[lab == hi_c].min()
            if lo >= hi:
                return None
            thr.append((lo + hi) / 2.0)
        cls_idx = np.zeros(4096, np.int64)
        for t in thr:
            cls_idx += s >= t
        if (classes[cls_idx] == lab).all():
            return thr
        return None

    w_int, thr = None, None
    for scale in (1000, 10_000, 100_000, 1_000_000, 8_000_000):
        cand = np.rint(w_real * scale)
        if np.abs(cand).max() * 12 >= 2 ** 24:       # keep f32-exact
            break
        got = try_weights(cand)
        if got is not None:
            w_int, thr = cand, got
            break
    if w_int is None:
        # max-margin LP fallback
        from scipy.optimize import linprog
        nv = 12 + len(classes)                        # w, thresholds..., margin
        A_ub, b_ub = [], []
        nthr = len(classes) - 1
        for i in range(4096):
            b = bits[i].astype(np.float64)
            ci = int(np.where(classes == lab[i])[0][0])
            if ci > 0:                                # s >= t_{ci-1} + m
                r = np.zeros(nv); r[:12] = -b; r[12 + ci - 1] = 1; r[-1] = 1
                A_ub.append(r); b_ub.append(0.0)
            if ci < nthr:                             # s <= t_{ci} - m
                r = np.zeros(nv); r[:12] = b; r[12 + ci] = -1; r[-1] = 1
                A_ub.append(r); b_ub.append(0.0)
        c_obj = np.zeros(nv); c_obj[-1] = -1.0
        bounds = [(-1, 1)] * 12 + [(None, None)] * nthr + [(0, None)]
        res = linprog(c_obj, A_ub=np.array(A_ub), b_ub=np.array(b_ub),
                      bounds=bounds, method="highs")
        if res.status != 0 or res.x[-1] <= 0:
            raise RuntimeError("no linear separator found")
        for scale in (1000, 10_000, 100_000, 1_000_000):
            cand = np.rint(res.x[:12] * scale)
            got = try_weights(cand)
            if got is not None:
                w_int, thr = cand, got
                break
        if w_int is None:
            raise RuntimeError("could not integerize separator")

    rows = [_out_row(c) for c in classes]
    base = rows[0]
    d1 = rows[1] - rows[0] if len(rows) > 1 else np.zeros(2 * L, np.float32)
    d2 = rows[2] - rows[1] if len(rows) > 2 else np.zeros(2 * L, np.float32)
    t1 = float(thr[0]) if len(thr) > 0 else 1e30
    t2 = float(thr[1]) if len(thr) > 1 else 1e30
    rows3 = np.stack([base, d1, d2]).astype(np.float32)              # [3, 48]
    return w_int.astype(np.int32), rows3, t1, t2


def _derive_tables(rows3):
    """l1-value tables a, b, c, na (24-wide each), exploiting l0 = -l1."""
    r = rows3.reshape(3, L, 2)
    if not np.array_equal(r[:, :, 0], -r[:, :, 1]):
        raise RuntimeError("output rows do not satisfy l0 == -l1")
    a, b, c = r[0, :, 1], r[1, :, 1], r[2, :, 1]                     # [24] each
    tab = np.concatenate([a, b, c, -a]).astype(ml_dtypes.bfloat16)   # [96]
    if not np.array_equal(tab.astype(np.float32),
                          np.concatenate([a, b, c, -a])):
        raise RuntimeError("table values not exact in bf16")
    return tab


# ----------------------------------------------------------------------------
# Device kernel
# ----------------------------------------------------------------------------

def _build_nc(t1, t2):
    """Raw-bass device program, hand-scheduled.

    Engine plan:
      SP:   w-const, in0, table-const, in1 DMA enqueues (in that order).
      Pool: per block, int32 dot -> reduce -> two threshold masks (narrow
            ops; runs ahead of DVE).
      DVE:  one doubling copy for the table, then per block the five wide
            bf16 ops (b*u1, c*u2, sum, +-a into the out tile).
      ACT:  per block, wait on DVE then start the out-DMA (HWDGE engine).

    The table ships from HBM replicated x TH; one doubling copy makes TH*2.
    All row-indexed tiles use column index t = rep*TH + th (rep in {0,1}),
    which is the natural packed t-order, so op views split t into (rep, th)
    with the table strided (r:TH, rep:NTAB*TH, th:1).
    """
    nc = bass.Bass()
    ids = nc.declare_dram_parameter("ids", [ROWS, L], I32, isOutput=False)
    wconst = nc.declare_dram_parameter("wconst", [12], I32, isOutput=False)
    tconst = nc.declare_dram_parameter("tconst", [NTAB * TH], BF16,
                                       isOutput=False)
    out = nc.declare_dram_parameter("out", [NBLK, 128, 2 * L * TB], BF16,
                                    isOutput=True)

    ids_v = ids.rearrange("(n p t) c -> n p (t c)", p=128, t=TB)   # [NBLK,128,TB*24]

    alu = mybir.AluOpType
    with ExitStack() as st:
        def sb(nm, shape, dt):
            return st.enter_context(nc.sbuf_tensor(nm, shape, dt))
        w_sb = sb("w_sb", [128, 12], I32)
        tab = sb("tab", [128, 2 * NTAB * TH], BF16)  # (rep, r, th)
        tins = [sb(f"tin{n}", [128, TB * L], I32) for n in range(NBLK)]
        prods = [sb(f"prod{n}", [128, TB * 12], I32) for n in range(NBLK)]
        halfs = [sb(f"half{n}", [128, TB * 6], I32) for n in range(NBLK)]
        scores = [sb(f"score{n}", [128, TB], I32) for n in range(NBLK)]
        u1s = [sb(f"u1_{n}", [128, TB], BF16) for n in range(NBLK)]
        u2s = [sb(f"u2_{n}", [128, TB], BF16) for n in range(NBLK)]
        vas = [sb(f"va{n}", [128, NV * TB], BF16) for n in range(NBLK)]
        vbs = [sb(f"vb{n}", [128, NV * TB], BF16) for n in range(NBLK)]
        vcs = [sb(f"vc{n}", [128, NV * TB], BF16) for n in range(NBLK)]
        otiles = [sb(f"ot{n}", [128, 2 * L * TB], BF16) for n in range(NBLK)]
        cw_sem = st.enter_context(nc.semaphore("cw_sem"))
        ct_sem = st.enter_context(nc.semaphore("ct_sem"))
        in_sems = [st.enter_context(nc.semaphore(f"in_sem{n}"))
                   for n in range(NBLK)]
        u_sem = st.enter_context(nc.semaphore("u_sem"))
        dve_sem = st.enter_context(nc.semaphore("dve_sem"))
        out_sem = st.enter_context(nc.semaphore("out_sem"))
        block = st.enter_context(nc.Block())

        # table views: r strided, t split into (rep, th)
        tab4 = tab[:, :].rearrange("p (rep r t) -> p r rep t", rep=2, t=TH)
        arep = tab4[:, 0 * NV:1 * NV]               # [128, 24, 2, TH]
        brep = tab4[:, 1 * NV:2 * NV]
        crep = tab4[:, 2 * NV:3 * NV]
        narep = tab4[:, 3 * NV:4 * NV]

        def vview(ts, n):                           # [128, NV, 2, TH]
            return ts[n][:, :].rearrange("p (l rep t) -> p l rep t",
                                         rep=2, t=TH)

        def uview(us, n):                           # [128, NV, 2, TH] bcast l
            return us[n][:, :].rearrange(
                "p (rep t) -> p rep t", rep=2).unsqueeze(1).broadcast_to(
                    [128, NV, 2, TH])

        @block.sync
        def _(sync):
            sync.dma_start(
                out=w_sb[:, :],
                in_=wconst[:].unsqueeze(0).broadcast_to([128, 12]),
            ).then_inc(cw_sem, 16)
            sync.dma_start(out=tins[0][:, :], in_=ids_v[0]).then_inc(
                in_sems[0], 16)
            sync.dma_start(
                out=tab[:, 0:NTAB * TH],
                in_=tconst[:].unsqueeze(0).broadcast_to([128, NTAB * TH]),
            ).then_inc(ct_sem, 16)
            for n in range(1, NBLK):
                sync.dma_start(out=tins[n][:, :], in_=ids_v[n]).then_inc(
                    in_sems[n], 16)

        @block.gpsimd
        def _(gpsimd):
            # Pool computes the int32 products and one pairwise pre-add;
            # the (free-axis) reduce is DVE-only.
            w_b = w_sb[:, :].unsqueeze(1).broadcast_to([128, TB, 12])
            gpsimd.wait_ge(cw_sem, 16)
            for n in range(NBLK):
                gpsimd.wait_ge(in_sems[n], 16)
                tv = tins[n][:, :].rearrange("p (t c) -> p t c", c=L)
                pv = prods[n][:, :].rearrange("p (t c) -> p t c", c=12)
                hv = halfs[n][:, :].rearrange("p (t c) -> p t c", c=6)
                with nc.allow_low_precision(reason="exact int32 dot"):
                    nc.gpsimd.tensor_tensor(
                        out=pv, in0=tv[:, :, 0:12], in1=w_b, op=alu.mult)
                    nc.gpsimd.tensor_tensor(
                        out=hv, in0=pv[:, :, 0:6], in1=pv[:, :, 6:12],
                        op=alu.add).then_inc(u_sem, 1)

        @block.scalar
        def _(scalar):
            for n in range(NBLK):
                scalar.wait_ge(dve_sem, n + 1)
                scalar.dma_start(out=out[n], in_=otiles[n][:, :]).then_inc(
                    out_sem, 16)
            scalar.wait_ge(out_sem, 16 * NBLK)

        @block.vector
        def _(vector):
            # DVE does not guarantee same-engine read-after-write consistency
            # between adjacent instructions (writes drain asynchronously):
            # every RAW pair below is separated by an unrelated wide op or an
            # explicit drain.
            def op_VA(n):
                nc.vector.tensor_tensor(out=vview(vas, n), in0=brep,
                                        in1=uview(u1s, n), op=alu.mult)

            def op_VB(n):
                nc.vector.tensor_tensor(out=vview(vbs, n), in0=crep,
                                        in1=uview(u2s, n), op=alu.mult)

            def op_VC(n):
                nc.vector.tensor_tensor(out=vview(vcs, n), in0=vview(vas, n),
                                        in1=vview(vbs, n), op=alu.add)

            def oview(n, j):                        # [128, NV, 2, TH]
                ov = otiles[n][:, :].rearrange(
                    "p (l j rep t) -> p l j rep t", j=2, rep=2, t=TH)
                return ov[:, :, j]

            def op_O1(n):
                nc.vector.tensor_tensor(out=oview(n, 1), in0=vview(vcs, n),
                                        in1=arep, op=alu.add)

            def op_O0(n):                           # l0 = -l1; signals ACT
                nc.vector.tensor_tensor(
                    out=oview(n, 0), in0=narep, in1=vview(vcs, n),
                    op=alu.subtract).then_inc(dve_sem, 1)

            def op_R(n):                            # reduce halves -> score
                hv = halfs[n][:, :].rearrange("p (t c) -> p t c", c=6)
                with nc.allow_low_precision(reason="exact int32 dot"):
                    nc.vector.tensor_reduce(
                        out=scores[n][:, :], in_=hv,
                        axis=mybir.AxisListType.X, op=alu.add)

            def op_U(n, us, thr):                   # threshold mask
                nc.vector.tensor_scalar(
                    out=us[n][:, :], in0=scores[n][:, :],
                    scalar1=thr, scalar2=None, op0=alu.is_ge)

            vector.wait_ge(ct_sem, 16)
            nc.vector.tensor_copy(                  # double the table: rep 1
                out=tab[:, NTAB * TH:2 * NTAB * TH],
                in_=tab[:, 0:NTAB * TH])
            vector.wait_ge(u_sem, 1)
            op_R(0)
            nc.vector.drain()                       # R0 -> U; also copy -> VA
            op_U(0, u1s, t1)
            op_U(0, u2s, t2)
            nc.vector.drain()                       # U -> VA0/VB0
            op_VA(0)
            op_VB(0)
            vector.wait_ge(u_sem, 2)
            op_R(1)                                 # separates VB0 -> VC0
            op_VC(0)
            op_U(1, u1s, t1)                        # R1 -> U1_1: VC0 between
            op_U(1, u2s, t2)
            op_O1(0)                                # VC0 -> O1_0: U,U between
            op_VA(1)                                # U1_1 -> VA1: U2,O1 betw.
            op_O0(0)
            op_VB(1)                                # U2_1 -> VB1: O1,VA,O0
            nc.vector.drain()                       # VB1 -> VC1
            op_VC(1)
            nc.vector.drain()                       # VC1 -> O1_1
            op_O1(1)
            op_O0(1)
    return nc


# ----------------------------------------------------------------------------
# Entry point
# ----------------------------------------------------------------------------

def kernel(**inputs):
    ids = np.ascontiguousarray(np.asarray(inputs["input_ids"], dtype=np.int32))
    assert ids.shape == (B_FULL, L), ids.shape
    w_int, rows3, t1, t2 = _derive_constants(
        *(np.asarray(inputs[k], dtype=np.float32)
          for k in ("emb", "W_v", "W_o", "W1", "b1", "W2", "b2"))
    )
    tab96 = _derive_tables(rows3)
    tconst = np.repeat(tab96, TH)                    # [NTAB*TH], (r, th) order
    nc = _build_nc(t1, t2)
    in_maps = [
        {"ids": ids[i * ROWS:(i + 1) * ROWS], "wconst": w_int,
         "tconst": tconst}
        for i in range(N_CORES)
    ]
    trace = bool(int(os.environ.get("BASSMUL_TRACE", "0")))
    try:
        res = run_bass_kernel_spmd(nc, in_maps, list(range(N_CORES)), trace=trace)
    except ModuleNotFoundError:
        # profiling hook unavailable in this environment; run untraced
        res = run_bass_kernel_spmd(nc, in_maps, list(range(N_CORES)), trace=False)
    _LAST["exec_time_ns"] = res.exec_time_ns
    _LAST["results"] = res
    parts = []
    for i in range(N_CORES):
        o = np.asarray(res.results[i]["out"])        # [NBLK, 128, 2*L*TB] bf16
        o = o.reshape(NBLK, 128, L, 2, 2, TH).transpose(0, 1, 4, 5, 2, 3)
        parts.append(o.reshape(ROWS, L, 2))
    return np.concatenate(parts, axis=0).astype(np.float32)


# revision 14
# speedup vs baseline: 1.0464x; 1.0464x over previous
"""Trainium2 kernel for nn_HandcraftedMultiplierV2.

Math notes (derived from the reference network's structure):
  - The attention stage collapses to a gather: the whole forward depends only
    on the 12 bits ids[b, 0:12].
  - For the actual parameter set the class total_int takes one of <=3
    consecutive values, reproduced exactly by an integer-weight linear
    threshold function of the bits (derived + verified over all 4096 patterns
    on the host at call time; integer arithmetic is exact in fp32 on device).
  - Output rows obey l0 = -l1 per position pair, and every output value
    ({0, +-0.5, +-9.5}) is exactly representable in bf16.

Device kernel (pure data parallel over 8 cores, t-last bf16 layout):
  score[b] = sum_i ids[b,i] * w_int[i]          (exact int32 dot, 12 cols)
  u1 = score >= T1, u2 = score >= T2            (bf16 0/1 masks, [128,TB])
  vc = b_tab*u1 + c_tab*u2                      (24-wide l1-value deltas)
  out[:, l, 1, t] = vc + a_tab                  (l1 values)
  out[:, l, 0, t] = na_tab - vc                 (l0 = -l1)
  All full-width ops are bf16 with packed innermost dims -> DVE 2x mode;
  output DMA is bf16 (half the bytes), host casts/transposes to f32 [B,L,2].
"""

import os
from contextlib import ExitStack

import numpy as np
import ml_dtypes

import concourse.bass as bass
import concourse.mybir as mybir
from concourse.bass_utils import run_bass_kernel_spmd

N_CORES = 8
B_FULL, L = 65536, 24
ROWS = B_FULL // N_CORES          # 8192 rows per core
TB = 16                           # batch rows per partition per block
TH = TB // 2                      # table replication shipped via DMA
NBLK = ROWS // (128 * TB)         # 4 blocks
NV = L                            # width of the l1-value (v) stage
NTAB = 4 * NV                     # a, b, c, na tables
F32 = mybir.dt.float32
BF16 = mybir.dt.bfloat16
I32 = mybir.dt.int32

_LAST = {}                        # exec_time_ns etc. for the test harness


# ----------------------------------------------------------------------------
# Host-side constant derivation (parameters only -- <10KB of data)
# ----------------------------------------------------------------------------

def _forward_totals(bits, emb, W_v, W_o, W1, b1, W2, b2):
    """fp32 `total` for each bit pattern, mirroring the reference arithmetic."""
    E = (emb.astype(np.float32) @ W_v.astype(np.float32).T)          # [2, 36]
    rep = np.repeat(np.arange(12), 3)                                # d -> head
    c = np.where(bits[:, rep] == 1, E[1][None, :], E[0][None, :]).astype(np.float32)
    attn = c @ W_o.astype(np.float32).T
    z = np.maximum(attn @ W1.astype(np.float32).T + b1.astype(np.float32), 0.0)
    mlp = z @ W2.astype(np.float32).T + b2.astype(np.float32)
    h2 = (attn + mlp).astype(np.float32)
    powers = np.exp2(np.arange(12)).astype(np.float32)
    return (h2[:, 12:24] * powers).sum(-1).astype(np.float32)


def _out_row(total_int):
    """The [L,2] output row for a given truncated total, flattened to [48]."""
    k = np.maximum(np.arange(L), 11) - 11
    ki = np.minimum(k, 11)
    m = k < 12
    bit = ((int(total_int) >> ki) & 1).astype(np.float32)
    l1 = np.where(m, bit * 10.0 - 0.5, 0.0)
    l0 = np.where(m, -bit * 10.0 + 0.5, 0.0)
    return np.stack([l0, l1], -1).reshape(2 * L).astype(np.float32)


def _derive_constants(emb, W_v, W_o, W1, b1, W2, b2):
    pat = np.arange(4096)
    bits = ((pat[:, None] >> np.arange(12)) & 1).astype(np.int64)    # [4096, 12]
    total = _forward_totals(bits, emb, W_v, W_o, W1, b1, W2, b2)
    lab = total.astype(np.int32)                                     # class per pattern
    classes = np.unique(lab)
    if len(classes) > 3:
        raise RuntimeError(f"expected <=3 classes, got {classes}")

    # Integer linear threshold reproducing `lab` exactly over all 4096 patterns.
    A = np.hstack([bits.astype(np.float64), np.ones((4096, 1))])
    coef, *_ = np.linalg.lstsq(A, total.astype(np.float64), rcond=None)
    w_real = coef[:12]

    def try_weights(w_int):
        s = bits @ w_int                                             # exact ints
        thr = []
        for lo_c, hi_c in zip(classes[:-1], classes[1:]):
            lo = s[lab == lo_c].max()
            hi = s[lab == hi_c].min()
            if lo >= hi:
                return None
            thr.append((lo + hi) / 2.0)
        cls_idx = np.zeros(4096, np.int64)
        for t in thr:
            cls_idx += s >= t
        if (classes[cls_idx] == lab).all():
            return thr
        return None

    w_int, thr = None, None
    for scale in (1000, 10_000, 100_000, 1_000_000, 8_000_000):
        cand = np.rint(w_real * scale)
        if np.abs(cand).max() * 12 >= 2 ** 24:       # keep f32-exact
            break
        got = try_weights(cand)
        if got is not None:
            w_int, thr = cand, got
            break
    if w_int is None:
        # max-margin LP fallback
        from scipy.optimize import linprog
        nv = 12 + len(classes)                        # w, thresholds..., margin
        A_ub, b_ub = [], []
        nthr = len(classes) - 1
        for i in range(4096):
            b = bits[i].astype(np.float64)
            ci = int(np.where(classes == lab[i])[0][0])
            if ci > 0:                                # s >= t_{ci-1} + m
                r = np.zeros(nv); r[:12] = -b; r[12 + ci - 1] = 1; r[-1] = 1
                A_ub.append(r); b_ub.append(0.0)
            if ci < nthr:                             # s <= t_{ci} - m
                r = np.zeros(nv); r[:12] = b; r[12 + ci] = -1; r[-1] = 1
                A_ub.append(r); b_ub.append(0.0)
        c_obj = np.zeros(nv); c_obj[-1] = -1.0
        bounds = [(-1, 1)] * 12 + [(None, None)] * nthr + [(0, None)]
        res = linprog(c_obj, A_ub=np.array(A_ub), b_ub=np.array(b_ub),
                      bounds=bounds, method="highs")
        if res.status != 0 or res.x[-1] <= 0:
            raise RuntimeError("no linear separator found")
        for scale in (1000, 10_000, 100_000, 1_000_000):
            cand = np.rint(res.x[:12] * scale)
            got = try_weights(cand)
            if got is not None:
                w_int, thr = cand, got
                break
        if w_int is None:
            raise RuntimeError("could not integerize separator")

    rows = [_out_row(c) for c in classes]
    base = rows[0]
    d1 = rows[1] - rows[0] if len(rows) > 1 else np.zeros(2 * L, np.float32)
    d2 = rows[2] - rows[1] if len(rows) > 2 else np.zeros(2 * L, np.float32)
    t1 = float(thr[0]) if len(thr) > 0 else 1e30
    t2 = float(thr[1]) if len(thr) > 1 else 1e30
    rows3 = np.stack([base, d1, d2]).astype(np.float32)              # [3, 48]
    return w_int.astype(np.int32), rows3, t1, t2


def _derive_tables(rows3):
    """l1-value tables a, b, c, na (24-wide each), exploiting l0 = -l1."""
    r = rows3.reshape(3, L, 2)
    if not np.array_equal(r[:, :, 0], -r[:, :, 1]):
        raise RuntimeError("output rows do not satisfy l0 == -l1")
    a, b, c = r[0, :, 1], r[1, :, 1], r[2, :, 1]                     # [24] each
    tab = np.concatenate([a, b, c, -a]).astype(ml_dtypes.bfloat16)   # [96]
    if not np.array_equal(tab.astype(np.float32),
                          np.concatenate([a, b, c, -a])):
        raise RuntimeError("table values not exact in bf16")
    return tab


# ----------------------------------------------------------------------------
# Device kernel
# ----------------------------------------------------------------------------

def _build_nc(t1, t2):
    """Raw-bass device program, hand-scheduled.

    Engine plan:
      SP:   in0, consts, in1..in3 DMA enqueues (all plain 128-row DMAs --
            per-partition-tiled consts avoid slow broadcast descriptors).
      Pool: per block, int32 products + one pairwise pre-add (Pool cannot do
            free-axis reduces); signals p_sem.
      DVE:  one doubling copy for the table, then per block: reduce ->
            threshold masks -> the five wide bf16 ops into the out tile.
            Blocks are software-pipelined with a stride-2 stagger so every
            same-engine RAW pair has an unrelated op in between (the DVE
            write pipeline does not guarantee RAW consistency for adjacent
            instructions); drains cover the prologue/epilogue.
      ACT:  per block, wait on DVE then start the out-DMA (HWDGE engine).

    All row-indexed tiles use column index t = rep*TH + th, so op views
    split t into (rep, th) with the table strided (r:TH, rep:NTAB*TH, th:1).
    """
    nc = bass.Bass()
    ids = nc.declare_dram_parameter("ids", [ROWS, L], I32, isOutput=False)
    consts = nc.declare_dram_parameter("consts", [128, 12 + NTAB * TH // 2],
                                       I32, isOutput=False)
    out = nc.declare_dram_parameter("out", [NBLK, 128, 2 * L * TB], BF16,
                                    isOutput=True)

    ids_v = ids.rearrange("(n p t) c -> n p (t c)", p=128, t=TB)

    alu = mybir.AluOpType
    with ExitStack() as st:
        def sb(nm, shape, dt):
            return st.enter_context(nc.sbuf_tensor(nm, shape, dt))
        cs = sb("cs", [128, 12 + NTAB * TH], I32)    # w | tab rep0 | rep1
        tins = [sb(f"tin{n}", [128, TB * L], I32) for n in range(NBLK)]
        prods = [sb(f"prod{n}", [128, TB * 12], I32) for n in range(NBLK)]
        halfs = [sb(f"half{n}", [128, TB * 6], I32) for n in range(NBLK)]
        scores = [sb(f"score{n}", [128, TB], I32) for n in range(NBLK)]
        u1s = [sb(f"u1_{n}", [128, TB], BF16) for n in range(NBLK)]
        u2s = [sb(f"u2_{n}", [128, TB], BF16) for n in range(NBLK)]
        vas = [sb(f"va{n}", [128, NV * TB], BF16) for n in range(NBLK)]
        vbs = [sb(f"vb{n}", [128, NV * TB], BF16) for n in range(NBLK)]
        vcs = [sb(f"vc{n}", [128, NV * TB], BF16) for n in range(NBLK)]
        otiles = [sb(f"ot{n}", [128, 2 * L * TB], BF16) for n in range(NBLK)]
        c_sem = st.enter_context(nc.semaphore("c_sem"))
        in_sems = [st.enter_context(nc.semaphore(f"in_sem{n}"))
                   for n in range(NBLK)]
        p_sem = st.enter_context(nc.semaphore("p_sem"))
        dve_sem = st.enter_context(nc.semaphore("dve_sem"))
        out_sem = st.enter_context(nc.semaphore("out_sem"))
        block = st.enter_context(nc.Block())

        HW = 12 + NTAB * TH // 2                     # i32 words in rep0 part
        tab4 = cs[:, 12:].bitcast(BF16).rearrange(
            "p (rep r t) -> p r rep t", rep=2, t=TH)  # [128, 96, 2, TH]
        arep = tab4[:, 0 * NV:1 * NV]
        brep = tab4[:, 1 * NV:2 * NV]
        crep = tab4[:, 2 * NV:3 * NV]
        narep = tab4[:, 3 * NV:4 * NV]

        def vview(ts, n):                            # [128, NV, 2, TH]
            return ts[n][:, :].rearrange("p (l rep t) -> p l rep t",
                                         rep=2, t=TH)

        def uview(us, n):
            return us[n][:, :].rearrange(
                "p (rep t) -> p rep t", rep=2).unsqueeze(1).broadcast_to(
                    [128, NV, 2, TH])

        @block.sync
        def _(sync):
            sync.dma_start(out=tins[0][:, :], in_=ids_v[0]).then_inc(
                in_sems[0], 16)
            sync.dma_start(out=cs[:, 0:HW], in_=consts[:, :]).then_inc(
                c_sem, 16)
            for n in range(1, NBLK):
                sync.dma_start(out=tins[n][:, :], in_=ids_v[n]).then_inc(
                    in_sems[n], 16)

        @block.gpsimd
        def _(gpsimd):
            w_b = cs[:, 0:12].unsqueeze(1).broadcast_to([128, TB, 12])
            gpsimd.wait_ge(c_sem, 16)
            for n in range(NBLK):
                gpsimd.wait_ge(in_sems[n], 16)
                tv = tins[n][:, :].rearrange("p (t c) -> p t c", c=L)
                pv = prods[n][:, :].rearrange("p (t c) -> p t c", c=12)
                hv = halfs[n][:, :].rearrange("p (t c) -> p t c", c=6)
                with nc.allow_low_precision(reason="exact int32 dot"):
                    nc.gpsimd.tensor_tensor(
                        out=pv, in0=tv[:, :, 0:12], in1=w_b, op=alu.mult)
                    nc.gpsimd.tensor_tensor(
                        out=hv, in0=pv[:, :, 0:6], in1=pv[:, :, 6:12],
                        op=alu.add).then_inc(p_sem, 1)

        @block.scalar
        def _(scalar):
            for n in range(NBLK):
                scalar.wait_ge(dve_sem, n + 1)
                scalar.dma_start(out=out[n], in_=otiles[n][:, :]).then_inc(
                    out_sem, 16)
            scalar.wait_ge(out_sem, 16 * NBLK)

        @block.vector
        def _(vector):
            def op_R(n):
                hv = halfs[n][:, :].rearrange("p (t c) -> p t c", c=6)
                with nc.allow_low_precision(reason="exact int32 dot"):
                    nc.vector.tensor_reduce(
                        out=scores[n][:, :], in_=hv,
                        axis=mybir.AxisListType.X, op=alu.add)

            def op_U(n, us, thr):
                nc.vector.tensor_scalar(
                    out=us[n][:, :], in0=scores[n][:, :],
                    scalar1=thr, scalar2=None, op0=alu.is_ge)

            def op_VA(n):
                nc.vector.tensor_tensor(out=vview(vas, n), in0=brep,
                                        in1=uview(u1s, n), op=alu.mult)

            def op_VB(n):
                nc.vector.tensor_tensor(out=vview(vbs, n), in0=crep,
                                        in1=uview(u2s, n), op=alu.mult)

            def op_VC(n):
                nc.vector.tensor_tensor(out=vview(vcs, n), in0=vview(vas, n),
                                        in1=vview(vbs, n), op=alu.add)

            def oview(n, j):
                ov = otiles[n][:, :].rearrange(
                    "p (l j rep t) -> p l j rep t", j=2, rep=2, t=TH)
                return ov[:, :, j]

            def op_O1(n):
                nc.vector.tensor_tensor(out=oview(n, 1), in0=vview(vcs, n),
                                        in1=arep, op=alu.add)

            def op_O0(n):                            # l0 = -l1; signals ACT
                nc.vector.tensor_tensor(
                    out=oview(n, 0), in0=narep, in1=vview(vcs, n),
                    op=alu.subtract).then_inc(dve_sem, 1)

            vector.wait_ge(c_sem, 16)
            nc.vector.tensor_copy(                   # double the table
                out=cs[:, HW:].bitcast(BF16),
                in_=cs[:, 12:HW].bitcast(BF16))
            vector.wait_ge(p_sem, 1)
            op_R(0)
            nc.vector.drain()                        # R0 -> U; copy -> VA0
            op_U(0, u1s, t1)
            op_U(0, u2s, t2)
            nc.vector.drain()                        # U -> VA0/VB0
            op_VA(0)
            op_VB(0)
            for n in range(1, NBLK):
                vector.wait_ge(p_sem, n + 1)
                op_R(n)                              # seps VB(n-1) -> VC(n-1)
                op_VC(n - 1)
                op_U(n, u1s, t1)                     # R(n) -> U: VC between
                op_U(n, u2s, t2)
                op_O1(n - 1)                         # VC -> O1: U,U between
                op_VA(n)                             # U1 -> VA: U2,O1 betw.
                op_O0(n - 1)
                op_VB(n)                             # U2 -> VB: O1,VA,O0
            nc.vector.drain()                        # VB(3) -> VC(3)
            op_VC(NBLK - 1)
            nc.vector.drain()                        # VC(3) -> O1(3)
            op_O1(NBLK - 1)
            op_O0(NBLK - 1)
    return nc


# ----------------------------------------------------------------------------
# Entry point
# ----------------------------------------------------------------------------

def kernel(**inputs):
    ids = np.ascontiguousarray(np.asarray(inputs["input_ids"], dtype=np.int32))
    assert ids.shape == (B_FULL, L), ids.shape
    w_int, rows3, t1, t2 = _derive_constants(
        *(np.asarray(inputs[k], dtype=np.float32)
          for k in ("emb", "W_v", "W_o", "W1", "b1", "W2", "b2"))
    )
    tab96 = _derive_tables(rows3)
    tab_rep = np.repeat(tab96, TH)                   # [NTAB*TH], (r, th) order
    crow = np.concatenate([w_int.view(np.uint8),
                           tab_rep.view(np.uint8)]).view(np.int32)
    consts = np.ascontiguousarray(
        np.broadcast_to(crow, (128, crow.size)).astype(np.int32))
    nc = _build_nc(t1, t2)
    in_maps = [
        {"ids": ids[i * ROWS:(i + 1) * ROWS], "consts": consts}
        for i in range(N_CORES)
    ]
    trace = bool(int(os.environ.get("BASSMUL_TRACE", "0")))
    try:
        res = run_bass_kernel_spmd(nc, in_maps, list(range(N_CORES)), trace=trace)
    except ModuleNotFoundError:
        # profiling hook unavailable in this environment; run untraced
        res = run_bass_kernel_spmd(nc, in_maps, list(range(N_CORES)), trace=False)
    _LAST["exec_time_ns"] = res.exec_time_ns
    _LAST["results"] = res
    parts = []
    for i in range(N_CORES):
        o = np.asarray(res.results[i]["out"])        # [NBLK, 128, 2*L*TB] bf16
        o = o.reshape(NBLK, 128, L, 2, 2, TH).transpose(0, 1, 4, 5, 2, 3)
        parts.append(o.reshape(ROWS, L, 2))
    return np.concatenate(parts, axis=0).astype(np.float32)
